# revision 30
# baseline (speedup 1.0000x reference)
"""ProteinMPNN-style message-passing layer on 8 Trainium2 NeuronCores.

Strategy (data-parallel over nodes, per the sharding hint):
  - B*N = 4096 nodes flattened; each of the 8 cores owns 512 consecutive
    nodes and their 512*48 edges.
  - On-chip activations are feature-major [H=128 partitions, tokens free];
    matmuls keep weights stationary (lhsT) and stream edge columns.
  - Neighbor features are fetched with SBUF-source dma_gather
    (transpose=True), which lands gathered rows directly in feature-major
    layout. The self (broadcast) term uses a stride-0 broadcast matmul rhs.
  - The K-sum of messages is moved before the third linear layer
    (sum_k gelu2 @ W3 == W3 applied to sum_k) to shrink that matmul 48x.
  - Updated node features are AllGathered mid-kernel (bf16 table) so the
    edge-update phase can gather from any node.
  - Edge-update output LayerNorm runs row-major (PE transpose + bn_stats),
    so h_E' stores row-major with full-size DMA bursts.

mask_V / mask_attend are ones per the problem spec (fill: "ones"); the
multiplies are identity and are skipped (asserted on the host).
"""

import numpy as np
import ml_dtypes

import concourse.bacc as bacc
import concourse.mybir as mybir
import concourse.tile as tile
from concourse.bass_utils import run_bass_kernel_spmd
from concourse.masks import make_identity

F32 = mybir.dt.float32
BF16 = mybir.dt.bfloat16
I16 = mybir.dt.int16
AF = mybir.ActivationFunctionType
ALU = mybir.AluOpType
AX = mybir.AxisListType

B, N, Kn, H = 2, 2048, 48, 128
NODES = B * N              # 4096
CORES = 8
PCN = NODES // CORES       # 512 nodes per core
E = PCN * Kn               # 24576 edges per core
SCALE = 30.0
EPS = 1e-5

CH = 384                   # edges per psum chunk (8 nodes * 48)
NPC = CH // Kn             # nodes per chunk (8)
NCH = E // CH              # 64 chunks
QT = E // 4                # gather quarter size (6144)
G1 = 2                     # phase-1 chunks per weight-group

bf16 = ml_dtypes.bfloat16

# wpack column offsets (bf16 weights, lhsT layout [in(contract) x out])
WOFF = {}
_c = 0
for _nm, _w in [("W1a", 128), ("W1b", 128), ("W1c", 128), ("W2", 128),
                ("W3", 128), ("W11a", 128), ("W11b", 128), ("W11c", 128),
                ("W12", 128), ("W13", 128), ("b13row", 128), ("Wdin", 512),
                ("Wdout0", 128), ("Wdout1", 128), ("Wdout2", 128), ("Wdout3", 128)]:
    WOFF[_nm] = _c
    _c += _w
WCOLS = _c

# bpack columns (f32): per-partition bias vectors
BOFF = {"b1": 0, "b2": 1, "b3s": 2, "b11": 3, "b12": 4, "bdout": 5,
        "bdin0": 6, "bdin1": 7, "bdin2": 8, "bdin3": 9}
NB = 10

# lnrep: replicated [128,128] f32 tiles: n1g n1b n2g n2b n3g n3b
LNOFF = {"n1g": 0, "n1b": 1, "n2g": 2, "n2b": 3, "n3g": 4, "n3b": 5}


def build_program(num_cores: int, nodes_total: int, trivial_ln3: bool,
                  trivial_ln12: bool, has_b13: bool = False):
    """Build the per-core Tile program. SPMD: all cores run the same code;
    per-core data differences come via in_maps."""
    nc = bacc.Bacc("TRN2", target_bir_lowering=False, debug=False,
                   num_devices=num_cores)
    NT = nodes_total
    NRANK = NT // 128          # table col-blocks

    d_hE32 = nc.dram_tensor("hE32", [128, E], F32, kind="ExternalInput").ap()
    d_hEbf = nc.dram_tensor("hEbf", [128, E], BF16, kind="ExternalInput").ap()
    d_hVfm32 = nc.dram_tensor("hVfm32", [128, PCN], F32, kind="ExternalInput").ap()
    d_hVfmbf = nc.dram_tensor("hVfmbf", [128, PCN], BF16, kind="ExternalInput").ap()
    d_table1 = nc.dram_tensor("table1", [128, NT], BF16, kind="ExternalInput").ap()
    d_idx = nc.dram_tensor("idx", [128, E // 16], I16, kind="ExternalInput").ap()
    d_wpack = nc.dram_tensor("wpack", [128, WCOLS], BF16, kind="ExternalInput").ap()
    d_bpack = nc.dram_tensor("bpack", [128, NB], F32, kind="ExternalInput").ap()
    d_lnrep = nc.dram_tensor("lnrep", [128, 6 * 128], F32, kind="ExternalInput").ap()

    import os
    DBG = bool(os.environ.get("MPNN_DEBUG"))
    if DBG:
        d_dbg_sg2 = nc.dram_tensor("dbg_sg2", [128, PCN], F32, kind="ExternalOutput").ap()
        d_dbg_r1 = nc.dram_tensor("dbg_r1", [128, PCN], F32, kind="ExternalOutput").ap()
        d_dbg_hv1 = nc.dram_tensor("dbg_hv1", [128, 4, 128], F32, kind="ExternalOutput").ap()
        d_dbg_r2 = nc.dram_tensor("dbg_r2", [128, PCN], F32, kind="ExternalOutput").ap()
        d_dbg_a2 = nc.dram_tensor("dbg_a2", [128, 768], F32, kind="ExternalOutput").ap()
        d_dbg_ps1 = nc.dram_tensor("dbg_ps1", [128, 768], F32, kind="ExternalOutput").ap()
        d_dbg_gnb = nc.dram_tensor("dbg_gnb", [128, 768], F32, kind="ExternalOutput").ap()
    d_hVout = nc.dram_tensor("hVout", [PCN, 128], F32, kind="ExternalOutput").ap()
    d_hEout = nc.dram_tensor("hEout", [E, 128], F32, kind="ExternalOutput").ap()

    multi = num_cores > 1
    if multi:
        d_ccin = nc.dram_tensor("cc_in", [128, 4 * 128], BF16).ap()
        d_ccout = nc.dram_tensor("cc_out", [num_cores * 128, 4 * 128], BF16,
                                 addr_space="Shared").ap()

    with tile.TileContext(nc) as tc:
        with tc.tile_pool(name="const", bufs=1) as cp, \
             tc.tile_pool(name="work", bufs=2) as wp, \
             tc.tile_pool(name="small", bufs=2) as sp:

            # ---- gather-gating loads first ----
            idx = cp.tile([128, E // 16], I16)
            nc.sync.dma_start(out=idx[:], in_=d_idx[:])
            def gather_quarters(src_ap, tag, pool):
                outs = []
                for q in range(4):
                    g = pool.tile([128, 1, QT], BF16, tag=f"{tag}{q % 2}")
                    nc.gpsimd.dma_gather(
                        out_ap=g[:], in_ap=src_ap,
                        idxs_ap=idx[:, q * (QT // 16):(q + 1) * (QT // 16)],
                        num_idxs=QT, num_idxs_reg=QT, elem_size=128,
                        transpose=True, single_packet=False,
                        sbuf_tokens_per_rank=128,
                        sbuf_free_dim_per_rank=256, sbuf_free_dim_pad_per_rank=0,
                        sbuf_byte_offset=0)
                    outs.append(g)
                return outs

            t1cm = tc.tile_pool(name="t1pool", bufs=1)
            t1pool = t1cm.__enter__()
            table1 = t1pool.tile([128, NT], BF16)
            nc.sync.dma_start(out=table1[:], in_=d_table1[:])
            gp1cm = tc.tile_pool(name="p1gath", bufs=2)
            gp1 = gp1cm.__enter__()
            gnb1 = gather_quarters(table1[:], "gnb1", gp1)
            hEbf_q = []
            for q in range(4):
                t = cp.tile([128, QT], BF16, tag=f"hEbf{q}")
                nc.sync.dma_start(out=t[:], in_=d_hEbf[:, q * QT:(q + 1) * QT])
                hEbf_q.append(t)
            # ---- constants ----
            W = cp.tile([128, WCOLS], BF16)
            nc.sync.dma_start(out=W[:], in_=d_wpack[:])
            bia = cp.tile([128, NB], F32)
            nc.sync.dma_start(out=bia[:], in_=d_bpack[:])
            lnrep = cp.tile([128, 6 * 128], F32)
            nc.sync.dma_start(out=lnrep[:], in_=d_lnrep[:])
            hVfm32 = cp.tile([128, PCN], F32)
            nc.sync.dma_start(out=hVfm32[:], in_=d_hVfm32[:])
            hVfmbf = cp.tile([128, PCN], BF16)
            nc.sync.dma_start(out=hVfmbf[:], in_=d_hVfmbf[:])
            iden = cp.tile([128, 128], BF16)
            make_identity(nc, iden[:])
            iden32 = cp.tile([128, 128], F32)
            make_identity(nc, iden32[:])
            cen32 = cp.tile([128, 128], F32)
            nc.gpsimd.memset(cen32[:], -1.0 / 128.0)
            nc.gpsimd.affine_select(
                out=cen32[:], in_=cen32[:], compare_op=ALU.not_equal,
                fill=127.0 / 128.0, base=0, pattern=[[-1, 128]],
                channel_multiplier=1)
            ones_row = cp.tile([1, 512], BF16)
            nc.gpsimd.memset(ones_row[:], 1.0)
            eps_col = cp.tile([128, 1], F32)
            nc.gpsimd.memset(eps_col[:], EPS)

            def w(name):
                return W[:, WOFF[name]:WOFF[name] + (512 if name == "Wdin" else 128)]

            def bcol(name):
                return bia[:, BOFF[name]:BOFF[name] + 1]

            def rep(name):
                o = LNOFF[name] * 128
                return lnrep[:, o:o + 128]

            def bn_combine(pool, bn6, T, tagp):
                """bn_stats [128,T,6] -> (m, inv) [128,T] f32.
                var4 = (cve+cvo)/32 + (me-mo)^2 = 4*var;
                inv  = 1/sqrt(0.25*var4 + eps) via ACT scale."""
                d = pool.tile([128, T], F32, tag=f"{tagp}d")
                v = pool.tile([128, T], F32, tag=f"{tagp}v")
                inv = pool.tile([128, T], F32, tag=f"{tagp}i")
                nc.vector.tensor_tensor(out=d[:], in0=bn6[:, :, 1],
                                        in1=bn6[:, :, 4], op=ALU.subtract)
                nc.vector.tensor_tensor(out=d[:], in0=d[:], in1=d[:], op=ALU.mult)
                nc.vector.tensor_tensor(out=v[:], in0=bn6[:, :, 2],
                                        in1=bn6[:, :, 5], op=ALU.add)
                nc.vector.tensor_scalar(out=v[:], in0=v[:], scalar1=1.0 / 32.0,
                                        scalar2=None, op0=ALU.mult)
                nc.vector.tensor_tensor(out=v[:], in0=v[:], in1=d[:], op=ALU.add)
                nc.scalar.activation(v[:], v[:], AF.Sqrt, bias=eps_col[:, 0:1],
                                     scale=0.25)
                nc.vector.reciprocal(inv[:], v[:])
                return inv

            Sg2 = cp.tile([128, PCN], F32)   # sum_k gelu2 accumulator

            # ================= phase 1: node update =================
            with tc.tile_pool(name="p1psA", bufs=2, space="PSUM") as psA, \
                 tc.tile_pool(name="p1psB", bufs=2, space="PSUM") as psB:

                for g in range(NCH // G1):
                    e0 = g * G1 * CH
                    n0 = e0 // Kn
                    nn = G1 * NPC
                    # bank-padded: each 384-wide chunk in its own 512-slot bank
                    ps1 = psA.tile([128, G1, 512], F32, tag="p1")
                    for c in range(G1):
                        eo = e0 + c * CH
                        q, qo = eo // QT, eo % QT
                        nc.tensor.matmul(ps1[:, c, 0:CH], w("W1b"),
                                         hEbf_q[q][:, qo:qo + CH],
                                         start=True, stop=False)
                    for c in range(G1):
                        eo = e0 + c * CH
                        q, qo = eo // QT, eo % QT
                        nc.tensor.matmul(ps1[:, c, 0:CH], w("W1c"),
                                         gnb1[q][:, 0, qo:qo + CH],
                                         start=False, stop=False)
                    for c in range(G1):
                        nc.tensor.matmul(
                            ps1[:, c, 0:CH].rearrange("p (n k) -> p n k", k=Kn),
                            w("W1a"),
                            hVfmbf[:, n0 + c * NPC:n0 + (c + 1) * NPC]
                            .to_broadcast([128, NPC, Kn]),
                            start=False, stop=True)
                    if DBG and g == 0:
                        dbg_ps1 = wp.tile([128, 768], F32, tag="dbgps1")
                        nc.vector.tensor_copy(
                            dbg_ps1[:].rearrange("p (c e) -> p c e", c=G1),
                            ps1[:, :, 0:CH])
                        nc.sync.dma_start(out=d_dbg_ps1[:], in_=dbg_ps1[:])
                        dbg_gnb = wp.tile([128, 768], F32, tag="dbggnb")
                        nc.vector.tensor_copy(dbg_gnb[:], gnb1[0][:, 0, 0:768])
                        nc.sync.dma_start(out=d_dbg_gnb[:], in_=dbg_gnb[:])
                    a1 = wp.tile([128, G1 * CH], BF16, tag="a1")
                    nc.scalar.activation(
                        a1[:].rearrange("p (c e) -> p c e", c=G1),
                        ps1[:, :, 0:CH], AF.Gelu, bias=bcol("b1"))
                    ps2 = psB.tile([128, G1, 512], F32, tag="p2")
                    for c in range(G1):
                        nc.tensor.matmul(ps2[:, c, 0:CH], w("W2"),
                                         a1[:, c * CH:(c + 1) * CH],
                                         start=True, stop=True)
                    a2 = wp.tile([128, G1 * CH], BF16, tag="a2")
                    nc.scalar.activation(
                        a2[:].rearrange("p (c e) -> p c e", c=G1),
                        ps2[:, :, 0:CH], AF.Gelu, bias=bcol("b2"))
                    nc.vector.tensor_reduce(
                        Sg2[:, n0:n0 + nn],
                        a2[:].rearrange("p (n k) -> p n k", k=Kn),
                        axis=AX.X, op=ALU.add)
                    if DBG and g == 0:
                        dbg_a2f = wp.tile([128, 768], F32, tag="dbga2")
                        nc.vector.tensor_copy(dbg_a2f[:], a2[:])
                        nc.sync.dma_start(out=d_dbg_a2[:], in_=dbg_a2f[:])

            gp1cm.__exit__(None, None, None)
            t1cm.__exit__(None, None, None)

            # ---- node path: dh, LN1, FFN, LN2 ----
            def ln_rowmajor_nodes(src_fm, gname, bname, out_rm_sb, pspool):
                ps_rm = pspool.tile([128, 4, 128], F32, tag="nrm")
                for j in range(4):
                    nc.tensor.matmul(ps_rm[:, j, :],
                                     src_fm[:, j * 128:(j + 1) * 128],
                                     cen32[:], start=True, stop=True)
                bn6 = sp.tile([128, 4, 8], F32, tag="nbn6")
                for j in range(4):
                    nc.vector.bn_stats(bn6[:, j, 0:6], ps_rm[:, j, :])
                inv = bn_combine(sp, bn6, 4, "n")
                for j in range(4):
                    nc.vector.tensor_scalar(
                        out=out_rm_sb[:, j, :], in0=ps_rm[:, j, :],
                        scalar1=inv[:, j:j + 1], scalar2=None, op0=ALU.mult)
                if not trivial_ln12:
                    nc.vector.tensor_tensor(
                        out=out_rm_sb[:], in0=out_rm_sb[:],
                        in1=rep(gname)[:].to_broadcast([128, 128, 4]).rearrange(
                            "p h j -> p j h"),
                        op=ALU.mult)
                    nc.vector.tensor_tensor(
                        out=out_rm_sb[:], in0=out_rm_sb[:],
                        in1=rep(bname)[:].to_broadcast([128, 128, 4]).rearrange(
                            "p h j -> p j h"),
                        op=ALU.add)

            with tc.tile_pool(name="npsA", bufs=1, space="PSUM") as npsA, \
                 tc.tile_pool(name="npsB", bufs=2, space="PSUM") as npsB:
                Sg2bf = cp.tile([128, PCN], BF16)
                nc.vector.tensor_copy(Sg2bf[:], Sg2[:])
                ps_dh = npsA.tile([128, PCN], F32, tag="pdh")
                nc.tensor.matmul(ps_dh[:], w("W3"), Sg2bf[:], start=True, stop=True)
                r1 = cp.tile([128, PCN], F32)
                nc.vector.tensor_scalar(out=r1[:], in0=ps_dh[:],
                                        scalar1=1.0 / SCALE, scalar2=bcol("b3s"),
                                        op0=ALU.mult, op1=ALU.add)
                nc.vector.tensor_tensor(out=r1[:], in0=r1[:], in1=hVfm32[:],
                                        op=ALU.add)

                hV1_rm = cp.tile([128, 4, 128], F32)
                ln_rowmajor_nodes(r1, "n1g", "n1b", hV1_rm, npsA)
                if DBG:
                    nc.sync.dma_start(out=d_dbg_sg2[:], in_=Sg2[:])
                    nc.sync.dma_start(out=d_dbg_r1[:], in_=r1[:])
                    nc.sync.dma_start(out=d_dbg_hv1[:], in_=hV1_rm[:])

                ps_fm = npsA.tile([128, PCN], F32, tag="nfm")
                for j in range(4):
                    nc.tensor.transpose(ps_fm[:, j * 128:(j + 1) * 128],
                                        hV1_rm[:, j, :], iden32[:])
                hV1fm32 = cp.tile([128, PCN], F32)
                nc.vector.tensor_copy(hV1fm32[:], ps_fm[:])
                hV1fmbf = cp.tile([128, PCN], BF16)
                nc.vector.tensor_copy(hV1fmbf[:], ps_fm[:])

                af = []
                for j in range(4):
                    psf = npsB.tile([128, PCN], F32, tag="ffn1")
                    nc.tensor.matmul(psf[:], w("Wdin")[:, j * 128:(j + 1) * 128],
                                     hV1fmbf[:], start=True, stop=True)
                    a = wp.tile([128, PCN], BF16, tag=f"af{j}")
                    nc.scalar.activation(a[:], psf[:], AF.Gelu,
                                         bias=bcol(f"bdin{j}"))
                    af.append(a)
                ps_o = npsA.tile([128, PCN], F32, tag="ffn2")
                for j in range(4):
                    nc.tensor.matmul(ps_o[:], w(f"Wdout{j}"), af[j][:],
                                     start=(j == 0), stop=(j == 3))
                r2 = cp.tile([128, PCN], F32)
                nc.vector.tensor_scalar_add(r2[:], ps_o[:], bcol("bdout"))
                nc.vector.tensor_tensor(out=r2[:], in0=r2[:], in1=hV1fm32[:],
                                        op=ALU.add)

                if DBG:
                    nc.sync.dma_start(out=d_dbg_r2[:], in_=r2[:])
                hV2_rm = cp.tile([128, 4, 128], F32)
                ln_rowmajor_nodes(r2, "n2g", "n2b", hV2_rm, npsA)

                nc.sync.dma_start(
                    out=d_hVout[:].rearrange("(t p) h -> p t h", p=128),
                    in_=hV2_rm[:])

                ps_fm2 = npsA.tile([128, PCN], F32, tag="nfm")
                for j in range(4):
                    nc.tensor.transpose(ps_fm2[:, j * 128:(j + 1) * 128],
                                        hV2_rm[:, j, :], iden32[:])
                hV2fmbf = cp.tile([128, PCN], BF16)
                nc.vector.tensor_copy(hV2fmbf[:], ps_fm2[:])

                tslice = cp.tile([128, 4, 128], BF16)
                nc.vector.tensor_copy(tslice[:], hV2_rm[:])

            # ---- phase-2 gather table via AllGather ----
            table2 = cp.tile([128, NRANK, 128], BF16)
            if multi:
                nc.sync.dma_start(out=d_ccin[:].rearrange("p (t h) -> p t h", t=4),
                                  in_=tslice[:])
                nc.gpsimd.collective_compute(
                    "AllGather", ALU.bypass,
                    ins=[d_ccin[:]], outs=[d_ccout[:]],
                    replica_groups=[list(range(num_cores))])
                nc.sync.dma_start(
                    out=table2[:].rearrange("p (c j) h -> p c j h",
                                            c=num_cores),
                    in_=d_ccout[:].rearrange("(c p) (j h) -> p c j h",
                                             p=128, h=128))
            else:
                nc.vector.tensor_copy(table2[:], tslice[:])

            # ================= phase 2: edge update =================
            with tc.tile_pool(name="p2gath", bufs=2) as gp2, \
                 tc.tile_pool(name="p2work", bufs=2) as wp2, \
                 tc.tile_pool(name="psq1", bufs=2, space="PSUM") as psq1, \
                 tc.tile_pool(name="psq2", bufs=2, space="PSUM") as psq2, \
                 tc.tile_pool(name="psq3", bufs=2, space="PSUM") as psq3, \
                 tc.tile_pool(name="psq4", bufs=2, space="PSUM") as psq4:

                gnb2 = gather_quarters(
                    table2[:].rearrange("p j h -> p (j h)"), "gnb2", gp2)

                GB = 8                      # chunks per tail batch
                NT3 = 3 * GB                # rm tiles per batch
                for gb in range(NCH // GB):
                    hE32g = wp2.tile([128, GB * CH], F32, tag="he32")
                    e00 = gb * GB * CH
                    nc.sync.dma_start(out=hE32g[:],
                                      in_=d_hE32[:, e00:e00 + GB * CH])
                    rm_sb = wp2.tile([128, NT3, 132], F32, tag="qrmsb")
                    bn6 = sp.tile([128, NT3, 8], F32, tag="qbn6")
                    for ci in range(GB):
                        c = gb * GB + ci
                        e0 = c * CH
                        eo = ci * CH
                        n0 = e0 // Kn
                        q, qo = e0 // QT, e0 % QT
                        ps1 = psq1.tile([128, CH], F32, tag="q1")
                        nc.tensor.matmul(ps1[:], w("W11b"),
                                         hEbf_q[q][:, qo:qo + CH],
                                         start=True, stop=False)
                        nc.tensor.matmul(ps1[:], w("W11c"),
                                         gnb2[q][:, 0, qo:qo + CH],
                                         start=False, stop=False)
                        nc.tensor.matmul(
                            ps1[:].rearrange("p (n k) -> p n k", k=Kn), w("W11a"),
                            hV2fmbf[:, n0:n0 + NPC].to_broadcast([128, NPC, Kn]),
                            start=False, stop=True)
                        a1 = wp.tile([128, CH], BF16, tag="qa1")
                        nc.scalar.activation(a1[:], ps1[:], AF.Gelu,
                                             bias=bcol("b11"))
                        ps2 = psq2.tile([128, CH], F32, tag="q2")
                        nc.tensor.matmul(ps2[:], w("W12"), a1[:],
                                         start=True, stop=True)
                        a2 = wp.tile([128, CH], BF16, tag="qa2")
                        nc.scalar.activation(a2[:], ps2[:], AF.Gelu,
                                             bias=bcol("b12"))
                        ps3 = psq3.tile([128, CH], F32, tag="q3")
                        nc.tensor.matmul(ps3[:], w("W13"), a2[:], start=True,
                                         stop=not has_b13)
                        if has_b13:
                            nc.tensor.matmul(
                                ps3[:],
                                W[0:1, WOFF["b13row"]:WOFF["b13row"] + 128],
                                ones_row[0:1, 0:CH], start=False, stop=True)

                        r = wp.tile([128, CH], F32, tag="qr")
                        nc.vector.tensor_tensor(out=r[:], in0=ps3[:],
                                                in1=hE32g[:, eo:eo + CH],
                                                op=ALU.add)
                        ps_rm = psq4.tile([128, 3, 128], F32, tag="qrm")
                        for t in range(3):
                            nc.tensor.matmul(ps_rm[:, t, :],
                                             r[:, t * 128:(t + 1) * 128],
                                             cen32[:], start=True, stop=True)
                        # copy rm to SBUF so psum frees and sqrt can batch
                        nc.scalar.copy(rm_sb[:, 3 * ci:3 * ci + 3, 0:128],
                                       ps_rm[:])
                        for t in range(3):
                            nc.vector.bn_stats(bn6[:, 3 * ci + t, 0:6],
                                               rm_sb[:, 3 * ci + t, 0:128])
                    inv = bn_combine(sp, bn6, NT3, "q")
                    out_rm = rm_sb
                    nc.vector.tensor_tensor(
                        out=out_rm[:, :, 0:128], in0=rm_sb[:, :, 0:128],
                        in1=inv[:].to_broadcast([128, NT3, 128]), op=ALU.mult)
                    if not trivial_ln3:
                        nc.vector.tensor_tensor(
                            out=out_rm[:, :, 0:128], in0=out_rm[:, :, 0:128],
                            in1=rep("n3g")[:].to_broadcast([128, 128, NT3])
                            .rearrange("p h t -> p t h"),
                            op=ALU.mult)
                        nc.vector.tensor_tensor(
                            out=out_rm[:, :, 0:128], in0=out_rm[:, :, 0:128],
                            in1=rep("n3b")[:].to_broadcast([128, 128, NT3])
                            .rearrange("p h t -> p t h"),
                            op=ALU.add)
                    nc.sync.dma_start(
                        out=d_hEout[e00:e00 + GB * CH, :]
                        .rearrange("(t p) h -> p t h", p=128),
                        in_=out_rm[:, :, 0:128])

    nc.compile()
    return nc


_PROG_CACHE = {}


def _get_prog(num_cores, nodes_total, trivial_ln3, trivial_ln12, has_b13):
    key = (num_cores, nodes_total, trivial_ln3, trivial_ln12, has_b13)
    if key not in _PROG_CACHE:
        _PROG_CACHE[key] = build_program(num_cores, nodes_total, trivial_ln3,
                                         trivial_ln12, has_b13)
    return _PROG_CACHE[key]


def make_in_maps(inputs, num_cores=CORES):
    """Host-side sharding/marshaling: layout transforms and casts only."""
    nodes = np.asarray(inputs["h_V"]).reshape(-1, H).shape[0]
    hV = np.asarray(inputs["h_V"], np.float32).reshape(nodes, H)
    hE = np.asarray(inputs["h_E"], np.float32).reshape(nodes, Kn, H)
    Eidx = np.asarray(inputs["E_idx"])
    bsz = Eidx.shape[0]
    npb = Eidx.shape[1]
    gidx = (Eidx.reshape(bsz, npb, Kn)
            + (np.arange(bsz) * npb)[:, None, None]).reshape(nodes, Kn)
    gidx = gidx.astype(np.int16)

    assert np.all(np.asarray(inputs["mask_V"]) == 1.0), "kernel assumes mask_V==1"
    assert np.all(np.asarray(inputs["mask_attend"]) == 1.0), \
        "kernel assumes mask_attend==1"

    W1 = np.asarray(inputs["W1"], np.float32)
    W11 = np.asarray(inputs["W11"], np.float32)
    wcols = np.zeros((128, WCOLS), np.float32)
    wcols[:, WOFF["W1a"]:WOFF["W1a"] + 128] = W1[0:128]
    wcols[:, WOFF["W1b"]:WOFF["W1b"] + 128] = W1[128:256]
    wcols[:, WOFF["W1c"]:WOFF["W1c"] + 128] = W1[256:384]
    wcols[:, WOFF["W2"]:WOFF["W2"] + 128] = np.asarray(inputs["W2"], np.float32)
    wcols[:, WOFF["W3"]:WOFF["W3"] + 128] = np.asarray(inputs["W3"], np.float32)
    wcols[:, WOFF["W11a"]:WOFF["W11a"] + 128] = W11[0:128]
    wcols[:, WOFF["W11b"]:WOFF["W11b"] + 128] = W11[128:256]
    wcols[:, WOFF["W11c"]:WOFF["W11c"] + 128] = W11[256:384]
    wcols[:, WOFF["W12"]:WOFF["W12"] + 128] = np.asarray(inputs["W12"], np.float32)
    wcols[:, WOFF["W13"]:WOFF["W13"] + 128] = np.asarray(inputs["W13"], np.float32)
    wcols[:, WOFF["Wdin"]:WOFF["Wdin"] + 512] = np.asarray(inputs["Wd_in"],
                                                           np.float32)
    wcols[0, WOFF["b13row"]:WOFF["b13row"] + 128] = np.asarray(inputs["b13"],
                                                               np.float32)
    Wd_out = np.asarray(inputs["Wd_out"], np.float32)
    for j in range(4):
        wcols[:, WOFF[f"Wdout{j}"]:WOFF[f"Wdout{j}"] + 128] = \
            Wd_out[j * 128:(j + 1) * 128]
    wpack = wcols.astype(bf16)

    bp = np.zeros((128, NB), np.float32)
    bp[:, BOFF["b1"]] = np.asarray(inputs["b1"], np.float32)
    bp[:, BOFF["b2"]] = np.asarray(inputs["b2"], np.float32)
    bp[:, BOFF["b3s"]] = np.asarray(inputs["b3"], np.float32) * Kn / SCALE
    bp[:, BOFF["b11"]] = np.asarray(inputs["b11"], np.float32)
    bp[:, BOFF["b12"]] = np.asarray(inputs["b12"], np.float32)
    bp[:, BOFF["bdout"]] = np.asarray(inputs["bd_out"], np.float32)
    bd_in = np.asarray(inputs["bd_in"], np.float32)
    for j in range(4):
        bp[:, BOFF[f"bdin{j}"]] = bd_in[j * 128:(j + 1) * 128]

    lnrep = np.zeros((128, 6 * 128), np.float32)
    for nm in ["n1g", "n1b", "n2g", "n2b", "n3g", "n3b"]:
        o = LNOFF[nm] * 128
        lnrep[:, o:o + 128] = np.asarray(inputs[nm], np.float32)[None, :]

    # phase-1 gather table: token t -> partition t%128, col block t//128
    table1 = hV.astype(bf16).reshape(nodes // 128, 128, H) \
        .transpose(1, 0, 2).reshape(128, nodes)

    in_maps = []
    pcn = nodes // num_cores
    for c in range(num_cores):
        n0 = c * pcn
        sl = slice(n0, n0 + pcn)
        hE_fm = np.ascontiguousarray(hE[sl].reshape(pcn * Kn, H).T)
        hv_slice = hV[sl].T
        idx_flat = gidx[sl].reshape(-1)
        idx_w = idx_flat.reshape(-1, 16).T
        idx_rep = np.tile(idx_w, (8, 1))
        in_maps.append({
            "hE32": hE_fm,
            "hEbf": hE_fm.astype(bf16),
            "hVfm32": np.ascontiguousarray(hv_slice, np.float32),
            "hVfmbf": np.ascontiguousarray(hv_slice).astype(bf16),
            "table1": np.ascontiguousarray(table1),
            "idx": np.ascontiguousarray(idx_rep),
            "wpack": wpack,
            "bpack": bp,
            "lnrep": lnrep,
        })
    return in_maps


def _trivial_flags(inputs):
    t3 = (np.all(np.asarray(inputs["n3g"]) == 1.0)
          and np.all(np.asarray(inputs["n3b"]) == 0.0))
    t12 = (np.all(np.asarray(inputs["n1g"]) == 1.0)
           and np.all(np.asarray(inputs["n1b"]) == 0.0)
           and np.all(np.asarray(inputs["n2g"]) == 1.0)
           and np.all(np.asarray(inputs["n2b"]) == 0.0))
    return t3, t12


def _run(inputs, trace=False):
    t3, t12 = _trivial_flags(inputs)
    has_b13 = bool(np.any(np.asarray(inputs["b13"]) != 0.0))
    nc = _get_prog(CORES, NODES, t3, t12, has_b13)
    in_maps = make_in_maps(inputs, CORES)
    res = run_bass_kernel_spmd(nc, in_maps, list(range(CORES)), trace=trace)
    hV_out = np.concatenate([res.results[c]["hVout"] for c in range(CORES)], 0)
    hE_out = np.concatenate([res.results[c]["hEout"] for c in range(CORES)], 0)
    out = (hV_out.reshape(B, N, H), hE_out.reshape(B, N, Kn, H))
    return out, res


def kernel(**inputs):
    return _run(inputs, trace=False)[0]


# revision 31
# speedup vs baseline: 1.1757x; 1.1757x over previous
"""ProteinMPNN-style message-passing layer on 8 Trainium2 NeuronCores.

Strategy (data-parallel over nodes, per the sharding hint):
  - B*N = 4096 nodes flattened; each of the 8 cores owns 512 consecutive
    nodes and their 512*48 edges.
  - On-chip activations are feature-major [H=128 partitions, tokens free];
    matmuls keep weights stationary (lhsT) and stream edge columns.
  - Neighbor features are fetched with SBUF-source dma_gather
    (transpose=True), which lands gathered rows directly in feature-major
    layout. The self (broadcast) term uses a stride-0 broadcast matmul rhs.
  - The K-sum of messages is moved before the third linear layer
    (sum_k gelu2 @ W3 == W3 applied to sum_k) to shrink that matmul 48x.
  - Updated node features are AllGathered mid-kernel (bf16 table) so the
    edge-update phase can gather from any node.
  - Edge-update output LayerNorm runs row-major (PE transpose + bn_stats),
    so h_E' stores row-major with full-size DMA bursts.

mask_V / mask_attend are ones per the problem spec (fill: "ones"); the
multiplies are identity and are skipped (asserted on the host).
"""

import numpy as np
import ml_dtypes

import concourse.bacc as bacc
import concourse.mybir as mybir
import concourse.tile as tile
from concourse.bass_utils import run_bass_kernel_spmd
from concourse.masks import make_identity

F32 = mybir.dt.float32
BF16 = mybir.dt.bfloat16
I16 = mybir.dt.int16
AF = mybir.ActivationFunctionType
ALU = mybir.AluOpType
AX = mybir.AxisListType

B, N, Kn, H = 2, 2048, 48, 128
NODES = B * N              # 4096
CORES = 8
PCN = NODES // CORES       # 512 nodes per core
E = PCN * Kn               # 24576 edges per core
SCALE = 30.0
EPS = 1e-5

CH = 384                   # edges per psum chunk (8 nodes * 48)
NPC = CH // Kn             # nodes per chunk (8)
NCH = E // CH              # 64 chunks
QT = E // 4                # gather quarter size (6144)
G1 = 2                     # phase-1 chunks per weight-group

bf16 = ml_dtypes.bfloat16

# wpack column offsets (bf16 weights, lhsT layout [in(contract) x out])
WOFF = {}
_c = 0
for _nm, _w in [("W1a", 128), ("W1b", 128), ("W1c", 128), ("W2", 128),
                ("W3", 128), ("W11a", 128), ("W11b", 128), ("W11c", 128),
                ("W12", 128), ("W13", 128), ("b13row", 128), ("Wdin", 512),
                ("Wdout0", 128), ("Wdout1", 128), ("Wdout2", 128), ("Wdout3", 128)]:
    WOFF[_nm] = _c
    _c += _w
WCOLS = _c

# bpack columns (f32): per-partition bias vectors
BOFF = {"b1": 0, "b2": 1, "b3s": 2, "b11": 3, "b12": 4, "bdout": 5,
        "bdin0": 6, "bdin1": 7, "bdin2": 8, "bdin3": 9}
NB = 10

# lnrep: replicated [128,128] f32 tiles: n1g n1b n2g n2b n3g n3b
LNOFF = {"n1g": 0, "n1b": 1, "n2g": 2, "n2b": 3, "n3g": 4, "n3b": 5}


def build_program(num_cores: int, nodes_total: int, trivial_ln3: bool,
                  trivial_ln12: bool, has_b13: bool = False):
    """Build the per-core Tile program. SPMD: all cores run the same code;
    per-core data differences come via in_maps."""
    nc = bacc.Bacc("TRN2", target_bir_lowering=False, debug=False,
                   num_devices=num_cores)
    NT = nodes_total
    NRANK = NT // 128          # table col-blocks

    d_hE32 = nc.dram_tensor("hE32", [128, E], F32, kind="ExternalInput").ap()
    d_hEbf = nc.dram_tensor("hEbf", [128, E], BF16, kind="ExternalInput").ap()
    d_hVfm32 = nc.dram_tensor("hVfm32", [128, PCN], F32, kind="ExternalInput").ap()
    d_hVfmbf = nc.dram_tensor("hVfmbf", [128, PCN], BF16, kind="ExternalInput").ap()
    d_table1 = nc.dram_tensor("table1", [128, NT], BF16, kind="ExternalInput").ap()
    d_idx = nc.dram_tensor("idx", [128, E // 16], I16, kind="ExternalInput").ap()
    d_wpack = nc.dram_tensor("wpack", [128, WCOLS], BF16, kind="ExternalInput").ap()
    d_bpack = nc.dram_tensor("bpack", [128, NB], F32, kind="ExternalInput").ap()
    d_lnrep = nc.dram_tensor("lnrep", [128, 6 * 128], F32, kind="ExternalInput").ap()

    import os
    DBG = bool(os.environ.get("MPNN_DEBUG"))
    if DBG:
        d_dbg_sg2 = nc.dram_tensor("dbg_sg2", [128, PCN], F32, kind="ExternalOutput").ap()
        d_dbg_r1 = nc.dram_tensor("dbg_r1", [128, PCN], F32, kind="ExternalOutput").ap()
        d_dbg_hv1 = nc.dram_tensor("dbg_hv1", [128, 4, 128], F32, kind="ExternalOutput").ap()
        d_dbg_r2 = nc.dram_tensor("dbg_r2", [128, PCN], F32, kind="ExternalOutput").ap()
        d_dbg_a2 = nc.dram_tensor("dbg_a2", [128, 768], F32, kind="ExternalOutput").ap()
        d_dbg_ps1 = nc.dram_tensor("dbg_ps1", [128, 768], F32, kind="ExternalOutput").ap()
        d_dbg_gnb = nc.dram_tensor("dbg_gnb", [128, 768], F32, kind="ExternalOutput").ap()
    d_hVout = nc.dram_tensor("hVout", [PCN, 128], F32, kind="ExternalOutput").ap()
    d_hEout = nc.dram_tensor("hEout", [E, 128], F32, kind="ExternalOutput").ap()

    multi = num_cores > 1
    if multi:
        d_ccin = nc.dram_tensor("cc_in", [128, 4 * 128], BF16).ap()
        d_ccout = nc.dram_tensor("cc_out", [num_cores * 128, 4 * 128], BF16,
                                 addr_space="Shared").ap()

    with tile.TileContext(nc) as tc:
        with tc.tile_pool(name="const", bufs=1) as cp, \
             tc.tile_pool(name="work", bufs=2) as wp, \
             tc.tile_pool(name="small", bufs=2) as sp:

            # ---- gather-gating loads first ----
            idx = cp.tile([128, E // 16], I16)
            nc.sync.dma_start(out=idx[:], in_=d_idx[:])
            def gather_quarters(src_ap, tag, pool):
                outs = []
                for q in range(4):
                    g = pool.tile([128, 1, QT], BF16, tag=f"{tag}{q % 2}")
                    nc.gpsimd.dma_gather(
                        out_ap=g[:], in_ap=src_ap,
                        idxs_ap=idx[:, q * (QT // 16):(q + 1) * (QT // 16)],
                        num_idxs=QT, num_idxs_reg=QT, elem_size=128,
                        transpose=True, single_packet=False,
                        sbuf_tokens_per_rank=128,
                        sbuf_free_dim_per_rank=256, sbuf_free_dim_pad_per_rank=0,
                        sbuf_byte_offset=0)
                    outs.append(g)
                return outs

            t1cm = tc.tile_pool(name="t1pool", bufs=1)
            t1pool = t1cm.__enter__()
            table1 = t1pool.tile([128, NT], BF16)
            nc.sync.dma_start(out=table1[:], in_=d_table1[:])
            hEbf_q = []
            for q in range(4):
                t = cp.tile([128, QT], BF16, tag=f"hEbf{q}")
                nc.sync.dma_start(out=t[:], in_=d_hEbf[:, q * QT:(q + 1) * QT])
                hEbf_q.append(t)
            # ---- constants ----
            W = cp.tile([128, WCOLS], BF16)
            nc.sync.dma_start(out=W[:], in_=d_wpack[:])
            bia = cp.tile([128, NB], F32)
            nc.sync.dma_start(out=bia[:], in_=d_bpack[:])
            lnrep = cp.tile([128, 6 * 128], F32)
            nc.sync.dma_start(out=lnrep[:], in_=d_lnrep[:])
            hVfm32 = cp.tile([128, PCN], F32)
            nc.sync.dma_start(out=hVfm32[:], in_=d_hVfm32[:])
            hVfmbf = cp.tile([128, PCN], BF16)
            nc.sync.dma_start(out=hVfmbf[:], in_=d_hVfmbf[:])
            iden = cp.tile([128, 128], BF16)
            make_identity(nc, iden[:])
            iden32 = cp.tile([128, 128], F32)
            make_identity(nc, iden32[:])
            cen32 = cp.tile([128, 128], F32)
            nc.gpsimd.memset(cen32[:], -1.0 / 128.0)
            nc.gpsimd.affine_select(
                out=cen32[:], in_=cen32[:], compare_op=ALU.not_equal,
                fill=127.0 / 128.0, base=0, pattern=[[-1, 128]],
                channel_multiplier=1)
            ones_row = cp.tile([1, 512], BF16)
            nc.gpsimd.memset(ones_row[:], 1.0)
            eps_col = cp.tile([128, 1], F32)
            nc.gpsimd.memset(eps_col[:], EPS)

            def w(name):
                return W[:, WOFF[name]:WOFF[name] + (512 if name == "Wdin" else 128)]

            def bcol(name):
                return bia[:, BOFF[name]:BOFF[name] + 1]

            def rep(name):
                o = LNOFF[name] * 128
                return lnrep[:, o:o + 128]

            def bn_combine(pool, bn6, T, tagp):
                """bn_stats [128,T,6] -> (m, inv) [128,T] f32.
                var4 = (cve+cvo)/32 + (me-mo)^2 = 4*var;
                inv  = 1/sqrt(0.25*var4 + eps) via ACT scale."""
                d = pool.tile([128, T], F32, tag=f"{tagp}d")
                v = pool.tile([128, T], F32, tag=f"{tagp}v")
                inv = pool.tile([128, T], F32, tag=f"{tagp}i")
                nc.vector.tensor_tensor(out=d[:], in0=bn6[:, :, 1],
                                        in1=bn6[:, :, 4], op=ALU.subtract)
                nc.vector.tensor_tensor(out=d[:], in0=d[:], in1=d[:], op=ALU.mult)
                nc.vector.tensor_tensor(out=v[:], in0=bn6[:, :, 2],
                                        in1=bn6[:, :, 5], op=ALU.add)
                nc.vector.tensor_scalar(out=v[:], in0=v[:], scalar1=1.0 / 32.0,
                                        scalar2=None, op0=ALU.mult)
                nc.vector.tensor_tensor(out=v[:], in0=v[:], in1=d[:], op=ALU.add)
                nc.scalar.activation(v[:], v[:], AF.Sqrt, bias=eps_col[:, 0:1],
                                     scale=0.25)
                nc.vector.reciprocal(inv[:], v[:])
                return inv

            Sg2 = cp.tile([128, PCN], F32)   # sum_k gelu2 accumulator

            # ================= phase 1: node update =================
            with tc.tile_pool(name="p1gath", bufs=2) as gp1, \
                 tc.tile_pool(name="p1psA", bufs=2, space="PSUM") as psA, \
                 tc.tile_pool(name="p1psB", bufs=2, space="PSUM") as psB:

                gnb1 = gather_quarters(table1[:], "gnb1", gp1)

                for g in range(NCH // G1):
                    e0 = g * G1 * CH
                    n0 = e0 // Kn
                    nn = G1 * NPC
                    # bank-padded: each 384-wide chunk in its own 512-slot bank
                    ps1 = psA.tile([128, G1, 512], F32, tag="p1")
                    for c in range(G1):
                        eo = e0 + c * CH
                        q, qo = eo // QT, eo % QT
                        nc.tensor.matmul(ps1[:, c, 0:CH], w("W1b"),
                                         hEbf_q[q][:, qo:qo + CH],
                                         start=True, stop=False)
                    for c in range(G1):
                        eo = e0 + c * CH
                        q, qo = eo // QT, eo % QT
                        nc.tensor.matmul(ps1[:, c, 0:CH], w("W1c"),
                                         gnb1[q][:, 0, qo:qo + CH],
                                         start=False, stop=False)
                    for c in range(G1):
                        nc.tensor.matmul(
                            ps1[:, c, 0:CH].rearrange("p (n k) -> p n k", k=Kn),
                            w("W1a"),
                            hVfmbf[:, n0 + c * NPC:n0 + (c + 1) * NPC]
                            .to_broadcast([128, NPC, Kn]),
                            start=False, stop=True)
                    if DBG and g == 0:
                        dbg_ps1 = wp.tile([128, 768], F32, tag="dbgps1")
                        nc.vector.tensor_copy(
                            dbg_ps1[:].rearrange("p (c e) -> p c e", c=G1),
                            ps1[:, :, 0:CH])
                        nc.sync.dma_start(out=d_dbg_ps1[:], in_=dbg_ps1[:])
                        dbg_gnb = wp.tile([128, 768], F32, tag="dbggnb")
                        nc.vector.tensor_copy(dbg_gnb[:], gnb1[0][:, 0, 0:768])
                        nc.sync.dma_start(out=d_dbg_gnb[:], in_=dbg_gnb[:])
                    a1 = wp.tile([128, G1 * CH], BF16, tag="a1")
                    nc.scalar.activation(
                        a1[:].rearrange("p (c e) -> p c e", c=G1),
                        ps1[:, :, 0:CH], AF.Gelu, bias=bcol("b1"))
                    ps2 = psB.tile([128, G1, 512], F32, tag="p2")
                    for c in range(G1):
                        nc.tensor.matmul(ps2[:, c, 0:CH], w("W2"),
                                         a1[:, c * CH:(c + 1) * CH],
                                         start=True, stop=True)
                    a2 = wp.tile([128, G1 * CH], BF16, tag="a2")
                    nc.scalar.activation(
                        a2[:].rearrange("p (c e) -> p c e", c=G1),
                        ps2[:, :, 0:CH], AF.Gelu, bias=bcol("b2"))
                    nc.vector.tensor_reduce(
                        Sg2[:, n0:n0 + nn],
                        a2[:].rearrange("p (n k) -> p n k", k=Kn),
                        axis=AX.X, op=ALU.add)
                    if DBG and g == 0:
                        dbg_a2f = wp.tile([128, 768], F32, tag="dbga2")
                        nc.vector.tensor_copy(dbg_a2f[:], a2[:])
                        nc.sync.dma_start(out=d_dbg_a2[:], in_=dbg_a2f[:])

            t1cm.__exit__(None, None, None)

            # ---- node path: dh, LN1, FFN, LN2 ----
            def ln_rowmajor_nodes(src_fm, gname, bname, out_rm_sb, pspool):
                ps_rm = pspool.tile([128, 4, 128], F32, tag="nrm")
                for j in range(4):
                    nc.tensor.matmul(ps_rm[:, j, :],
                                     src_fm[:, j * 128:(j + 1) * 128],
                                     cen32[:], start=True, stop=True)
                bn6 = sp.tile([128, 4, 8], F32, tag="nbn6")
                for j in range(4):
                    nc.vector.bn_stats(bn6[:, j, 0:6], ps_rm[:, j, :])
                inv = bn_combine(sp, bn6, 4, "n")
                for j in range(4):
                    nc.vector.tensor_scalar(
                        out=out_rm_sb[:, j, :], in0=ps_rm[:, j, :],
                        scalar1=inv[:, j:j + 1], scalar2=None, op0=ALU.mult)
                if not trivial_ln12:
                    nc.vector.tensor_tensor(
                        out=out_rm_sb[:], in0=out_rm_sb[:],
                        in1=rep(gname)[:].to_broadcast([128, 128, 4]).rearrange(
                            "p h j -> p j h"),
                        op=ALU.mult)
                    nc.vector.tensor_tensor(
                        out=out_rm_sb[:], in0=out_rm_sb[:],
                        in1=rep(bname)[:].to_broadcast([128, 128, 4]).rearrange(
                            "p h j -> p j h"),
                        op=ALU.add)

            with tc.tile_pool(name="npsA", bufs=1, space="PSUM") as npsA, \
                 tc.tile_pool(name="npsB", bufs=2, space="PSUM") as npsB:
                Sg2bf = cp.tile([128, PCN], BF16)
                nc.vector.tensor_copy(Sg2bf[:], Sg2[:])
                ps_dh = npsA.tile([128, PCN], F32, tag="pdh")
                nc.tensor.matmul(ps_dh[:], w("W3"), Sg2bf[:], start=True, stop=True)
                r1 = cp.tile([128, PCN], F32)
                nc.vector.tensor_scalar(out=r1[:], in0=ps_dh[:],
                                        scalar1=1.0 / SCALE, scalar2=bcol("b3s"),
                                        op0=ALU.mult, op1=ALU.add)
                nc.vector.tensor_tensor(out=r1[:], in0=r1[:], in1=hVfm32[:],
                                        op=ALU.add)

                hV1_rm = cp.tile([128, 4, 128], F32)
                ln_rowmajor_nodes(r1, "n1g", "n1b", hV1_rm, npsA)
                if DBG:
                    nc.sync.dma_start(out=d_dbg_sg2[:], in_=Sg2[:])
                    nc.sync.dma_start(out=d_dbg_r1[:], in_=r1[:])
                    nc.sync.dma_start(out=d_dbg_hv1[:], in_=hV1_rm[:])

                ps_fm = npsA.tile([128, PCN], F32, tag="nfm")
                for j in range(4):
                    nc.tensor.transpose(ps_fm[:, j * 128:(j + 1) * 128],
                                        hV1_rm[:, j, :], iden32[:])
                hV1fm32 = cp.tile([128, PCN], F32)
                nc.vector.tensor_copy(hV1fm32[:], ps_fm[:])
                hV1fmbf = cp.tile([128, PCN], BF16)
                nc.vector.tensor_copy(hV1fmbf[:], ps_fm[:])

                af = []
                for j in range(4):
                    psf = npsB.tile([128, PCN], F32, tag="ffn1")
                    nc.tensor.matmul(psf[:], w("Wdin")[:, j * 128:(j + 1) * 128],
                                     hV1fmbf[:], start=True, stop=True)
                    a = wp.tile([128, PCN], BF16, tag=f"af{j}")
                    nc.scalar.activation(a[:], psf[:], AF.Gelu,
                                         bias=bcol(f"bdin{j}"))
                    af.append(a)
                ps_o = npsA.tile([128, PCN], F32, tag="ffn2")
                for j in range(4):
                    nc.tensor.matmul(ps_o[:], w(f"Wdout{j}"), af[j][:],
                                     start=(j == 0), stop=(j == 3))
                r2 = cp.tile([128, PCN], F32)
                nc.vector.tensor_scalar_add(r2[:], ps_o[:], bcol("bdout"))
                nc.vector.tensor_tensor(out=r2[:], in0=r2[:], in1=hV1fm32[:],
                                        op=ALU.add)

                if DBG:
                    nc.sync.dma_start(out=d_dbg_r2[:], in_=r2[:])
                hV2_rm = cp.tile([128, 4, 128], F32)
                ln_rowmajor_nodes(r2, "n2g", "n2b", hV2_rm, npsA)

                nc.sync.dma_start(
                    out=d_hVout[:].rearrange("(t p) h -> p t h", p=128),
                    in_=hV2_rm[:])

                ps_fm2 = npsA.tile([128, PCN], F32, tag="nfm")
                for j in range(4):
                    nc.tensor.transpose(ps_fm2[:, j * 128:(j + 1) * 128],
                                        hV2_rm[:, j, :], iden32[:])
                hV2fmbf = cp.tile([128, PCN], BF16)
                nc.vector.tensor_copy(hV2fmbf[:], ps_fm2[:])

                tslice = cp.tile([128, 4, 128], BF16)
                nc.vector.tensor_copy(tslice[:], hV2_rm[:])

            # ---- phase-2 gather table via AllGather ----
            table2 = cp.tile([128, NRANK, 128], BF16)
            if multi:
                nc.sync.dma_start(out=d_ccin[:].rearrange("p (t h) -> p t h", t=4),
                                  in_=tslice[:])
                nc.gpsimd.collective_compute(
                    "AllGather", ALU.bypass,
                    ins=[d_ccin[:]], outs=[d_ccout[:]],
                    replica_groups=[list(range(num_cores))])
                nc.sync.dma_start(
                    out=table2[:].rearrange("p (c j) h -> p c j h",
                                            c=num_cores),
                    in_=d_ccout[:].rearrange("(c p) (j h) -> p c j h",
                                             p=128, h=128))
            else:
                nc.vector.tensor_copy(table2[:], tslice[:])

            # ================= phase 2: edge update =================
            with tc.tile_pool(name="p2gath", bufs=2) as gp2, \
                 tc.tile_pool(name="p2work", bufs=2) as wp2, \
                 tc.tile_pool(name="psq1", bufs=2, space="PSUM") as psq1, \
                 tc.tile_pool(name="psq2", bufs=2, space="PSUM") as psq2, \
                 tc.tile_pool(name="psq3", bufs=2, space="PSUM") as psq3, \
                 tc.tile_pool(name="psq4", bufs=2, space="PSUM") as psq4:

                gnb2 = gather_quarters(
                    table2[:].rearrange("p j h -> p (j h)"), "gnb2", gp2)

                GB = 8                      # chunks per tail batch
                NT3 = 3 * GB                # rm tiles per batch
                for gb in range(NCH // GB):
                    hE32g = wp2.tile([128, GB * CH], F32, tag="he32")
                    e00 = gb * GB * CH
                    nc.sync.dma_start(out=hE32g[:],
                                      in_=d_hE32[:, e00:e00 + GB * CH])
                    rm_sb = wp2.tile([128, NT3, 128], F32, tag="qrmsb")
                    for ci in range(GB):
                        c = gb * GB + ci
                        e0 = c * CH
                        eo = ci * CH
                        n0 = e0 // Kn
                        q, qo = e0 // QT, e0 % QT
                        ps1 = psq1.tile([128, CH], F32, tag="q1")
                        nc.tensor.matmul(ps1[:], w("W11b"),
                                         hEbf_q[q][:, qo:qo + CH],
                                         start=True, stop=False)
                        nc.tensor.matmul(ps1[:], w("W11c"),
                                         gnb2[q][:, 0, qo:qo + CH],
                                         start=False, stop=False)
                        nc.tensor.matmul(
                            ps1[:].rearrange("p (n k) -> p n k", k=Kn), w("W11a"),
                            hV2fmbf[:, n0:n0 + NPC].to_broadcast([128, NPC, Kn]),
                            start=False, stop=True)
                        a1 = wp.tile([128, CH], BF16, tag="qa1")
                        nc.scalar.activation(a1[:], ps1[:], AF.Gelu,
                                             bias=bcol("b11"))
                        ps2 = psq2.tile([128, CH], F32, tag="q2")
                        nc.tensor.matmul(ps2[:], w("W12"), a1[:],
                                         start=True, stop=True)
                        a2 = wp.tile([128, CH], BF16, tag="qa2")
                        nc.scalar.activation(a2[:], ps2[:], AF.Gelu,
                                             bias=bcol("b12"))
                        ps3 = psq3.tile([128, CH], F32, tag="q3")
                        nc.tensor.matmul(ps3[:], w("W13"), a2[:], start=True,
                                         stop=not has_b13)
                        if has_b13:
                            nc.tensor.matmul(
                                ps3[:],
                                W[0:1, WOFF["b13row"]:WOFF["b13row"] + 128],
                                ones_row[0:1, 0:CH], start=False, stop=True)

                        r = wp.tile([128, CH], F32, tag="qr")
                        nc.vector.tensor_tensor(out=r[:], in0=ps3[:],
                                                in1=hE32g[:, eo:eo + CH],
                                                op=ALU.add)
                        ps_rm = psq4.tile([128, 3, 128], F32, tag="qrm")
                        for t in range(3):
                            nc.tensor.matmul(ps_rm[:, t, :],
                                             r[:, t * 128:(t + 1) * 128],
                                             cen32[:], start=True, stop=True)
                        # copy rm to SBUF so psum frees and sqrt can batch
                        nc.scalar.copy(rm_sb[:, 3 * ci:3 * ci + 3, :], ps_rm[:])
                    bn6 = sp.tile([128, NT3, 8], F32, tag="qbn6")
                    for t in range(NT3):
                        nc.vector.bn_stats(bn6[:, t, 0:6], rm_sb[:, t, :])
                    inv = bn_combine(sp, bn6, NT3, "q")
                    out_rm = rm_sb
                    nc.vector.tensor_tensor(
                        out=out_rm[:], in0=rm_sb[:],
                        in1=inv[:].to_broadcast([128, NT3, 128]), op=ALU.mult)
                    if not trivial_ln3:
                        nc.vector.tensor_tensor(
                            out=out_rm[:], in0=out_rm[:],
                            in1=rep("n3g")[:].to_broadcast([128, 128, NT3])
                            .rearrange("p h t -> p t h"),
                            op=ALU.mult)
                        nc.vector.tensor_tensor(
                            out=out_rm[:], in0=out_rm[:],
                            in1=rep("n3b")[:].to_broadcast([128, 128, NT3])
                            .rearrange("p h t -> p t h"),
                            op=ALU.add)
                    nc.sync.dma_start(
                        out=d_hEout[e00:e00 + GB * CH, :]
                        .rearrange("(t p) h -> p t h", p=128),
                        in_=out_rm[:])

    nc.compile()
    return nc


_PROG_CACHE = {}


def _get_prog(num_cores, nodes_total, trivial_ln3, trivial_ln12, has_b13):
    key = (num_cores, nodes_total, trivial_ln3, trivial_ln12, has_b13)
    if key not in _PROG_CACHE:
        _PROG_CACHE[key] = build_program(num_cores, nodes_total, trivial_ln3,
                                         trivial_ln12, has_b13)
    return _PROG_CACHE[key]


def make_in_maps(inputs, num_cores=CORES):
    """Host-side sharding/marshaling: layout transforms and casts only."""
    nodes = np.asarray(inputs["h_V"]).reshape(-1, H).shape[0]
    hV = np.asarray(inputs["h_V"], np.float32).reshape(nodes, H)
    hE = np.asarray(inputs["h_E"], np.float32).reshape(nodes, Kn, H)
    Eidx = np.asarray(inputs["E_idx"])
    bsz = Eidx.shape[0]
    npb = Eidx.shape[1]
    gidx = (Eidx.reshape(bsz, npb, Kn)
            + (np.arange(bsz) * npb)[:, None, None]).reshape(nodes, Kn)
    gidx = gidx.astype(np.int16)

    assert np.all(np.asarray(inputs["mask_V"]) == 1.0), "kernel assumes mask_V==1"
    assert np.all(np.asarray(inputs["mask_attend"]) == 1.0), \
        "kernel assumes mask_attend==1"

    W1 = np.asarray(inputs["W1"], np.float32)
    W11 = np.asarray(inputs["W11"], np.float32)
    wcols = np.zeros((128, WCOLS), np.float32)
    wcols[:, WOFF["W1a"]:WOFF["W1a"] + 128] = W1[0:128]
    wcols[:, WOFF["W1b"]:WOFF["W1b"] + 128] = W1[128:256]
    wcols[:, WOFF["W1c"]:WOFF["W1c"] + 128] = W1[256:384]
    wcols[:, WOFF["W2"]:WOFF["W2"] + 128] = np.asarray(inputs["W2"], np.float32)
    wcols[:, WOFF["W3"]:WOFF["W3"] + 128] = np.asarray(inputs["W3"], np.float32)
    wcols[:, WOFF["W11a"]:WOFF["W11a"] + 128] = W11[0:128]
    wcols[:, WOFF["W11b"]:WOFF["W11b"] + 128] = W11[128:256]
    wcols[:, WOFF["W11c"]:WOFF["W11c"] + 128] = W11[256:384]
    wcols[:, WOFF["W12"]:WOFF["W12"] + 128] = np.asarray(inputs["W12"], np.float32)
    wcols[:, WOFF["W13"]:WOFF["W13"] + 128] = np.asarray(inputs["W13"], np.float32)
    wcols[:, WOFF["Wdin"]:WOFF["Wdin"] + 512] = np.asarray(inputs["Wd_in"],
                                                           np.float32)
    wcols[0, WOFF["b13row"]:WOFF["b13row"] + 128] = np.asarray(inputs["b13"],
                                                               np.float32)
    Wd_out = np.asarray(inputs["Wd_out"], np.float32)
    for j in range(4):
        wcols[:, WOFF[f"Wdout{j}"]:WOFF[f"Wdout{j}"] + 128] = \
            Wd_out[j * 128:(j + 1) * 128]
    wpack = wcols.astype(bf16)

    bp = np.zeros((128, NB), np.float32)
    bp[:, BOFF["b1"]] = np.asarray(inputs["b1"], np.float32)
    bp[:, BOFF["b2"]] = np.asarray(inputs["b2"], np.float32)
    bp[:, BOFF["b3s"]] = np.asarray(inputs["b3"], np.float32) * Kn / SCALE
    bp[:, BOFF["b11"]] = np.asarray(inputs["b11"], np.float32)
    bp[:, BOFF["b12"]] = np.asarray(inputs["b12"], np.float32)
    bp[:, BOFF["bdout"]] = np.asarray(inputs["bd_out"], np.float32)
    bd_in = np.asarray(inputs["bd_in"], np.float32)
    for j in range(4):
        bp[:, BOFF[f"bdin{j}"]] = bd_in[j * 128:(j + 1) * 128]

    lnrep = np.zeros((128, 6 * 128), np.float32)
    for nm in ["n1g", "n1b", "n2g", "n2b", "n3g", "n3b"]:
        o = LNOFF[nm] * 128
        lnrep[:, o:o + 128] = np.asarray(inputs[nm], np.float32)[None, :]

    # phase-1 gather table: token t -> partition t%128, col block t//128
    table1 = hV.astype(bf16).reshape(nodes // 128, 128, H) \
        .transpose(1, 0, 2).reshape(128, nodes)

    in_maps = []
    pcn = nodes // num_cores
    for c in range(num_cores):
        n0 = c * pcn
        sl = slice(n0, n0 + pcn)
        hE_fm = np.ascontiguousarray(hE[sl].reshape(pcn * Kn, H).T)
        hv_slice = hV[sl].T
        idx_flat = gidx[sl].reshape(-1)
        idx_w = idx_flat.reshape(-1, 16).T
        idx_rep = np.tile(idx_w, (8, 1))
        in_maps.append({
            "hE32": hE_fm,
            "hEbf": hE_fm.astype(bf16),
            "hVfm32": np.ascontiguousarray(hv_slice, np.float32),
            "hVfmbf": np.ascontiguousarray(hv_slice).astype(bf16),
            "table1": np.ascontiguousarray(table1),
            "idx": np.ascontiguousarray(idx_rep),
            "wpack": wpack,
            "bpack": bp,
            "lnrep": lnrep,
        })
    return in_maps


def _trivial_flags(inputs):
    t3 = (np.all(np.asarray(inputs["n3g"]) == 1.0)
          and np.all(np.asarray(inputs["n3b"]) == 0.0))
    t12 = (np.all(np.asarray(inputs["n1g"]) == 1.0)
           and np.all(np.asarray(inputs["n1b"]) == 0.0)
           and np.all(np.asarray(inputs["n2g"]) == 1.0)
           and np.all(np.asarray(inputs["n2b"]) == 0.0))
    return t3, t12


def _run(inputs, trace=False):
    t3, t12 = _trivial_flags(inputs)
    has_b13 = bool(np.any(np.asarray(inputs["b13"]) != 0.0))
    nc = _get_prog(CORES, NODES, t3, t12, has_b13)
    in_maps = make_in_maps(inputs, CORES)
    res = run_bass_kernel_spmd(nc, in_maps, list(range(CORES)), trace=trace)
    hV_out = np.concatenate([res.results[c]["hVout"] for c in range(CORES)], 0)
    hE_out = np.concatenate([res.results[c]["hEout"] for c in range(CORES)], 0)
    out = (hV_out.reshape(B, N, H), hE_out.reshape(B, N, Kn, H))
    return out, res


def kernel(**inputs):
    return _run(inputs, trace=False)[0]


# revision 34
# speedup vs baseline: 1.3071x; 1.1117x over previous
"""ProteinMPNN-style message-passing layer on 8 Trainium2 NeuronCores.

Strategy (data-parallel over nodes, per the sharding hint):
  - B*N = 4096 nodes flattened; each of the 8 cores owns 512 consecutive
    nodes and their 512*48 edges.
  - On-chip activations are feature-major [H=128 partitions, tokens free];
    matmuls keep weights stationary (lhsT) and stream edge columns.
  - Neighbor features are fetched with SBUF-source dma_gather
    (transpose=True), which lands gathered rows directly in feature-major
    layout. The self (broadcast) term uses a stride-0 broadcast matmul rhs.
  - The K-sum of messages is moved before the third linear layer
    (sum_k gelu2 @ W3 == W3 applied to sum_k) to shrink that matmul 48x.
  - Updated node features are AllGathered mid-kernel (bf16 table) so the
    edge-update phase can gather from any node.
  - Edge-update output LayerNorm runs row-major (PE transpose + bn_stats),
    so h_E' stores row-major with full-size DMA bursts.

mask_V / mask_attend are ones per the problem spec (fill: "ones"); the
multiplies are identity and are skipped (asserted on the host).
"""

import numpy as np
import ml_dtypes

import concourse.bacc as bacc
import concourse.mybir as mybir
import concourse.tile as tile
from concourse.bass_utils import run_bass_kernel_spmd
from concourse.masks import make_identity

F32 = mybir.dt.float32
BF16 = mybir.dt.bfloat16
I16 = mybir.dt.int16
AF = mybir.ActivationFunctionType
ALU = mybir.AluOpType
AX = mybir.AxisListType

B, N, Kn, H = 2, 2048, 48, 128
NODES = B * N              # 4096
CORES = 8
PCN = NODES // CORES       # 512 nodes per core
E = PCN * Kn               # 24576 edges per core
SCALE = 30.0
EPS = 1e-5

CH = 384                   # edges per psum chunk (8 nodes * 48)
NPC = CH // Kn             # nodes per chunk (8)
NCH = E // CH              # 64 chunks
NSPLIT = 8
QT = E // NSPLIT           # gather split size (3072)
G1 = 2                     # phase-1 chunks per weight-group

bf16 = ml_dtypes.bfloat16

# wpack column offsets (bf16 weights, lhsT layout [in(contract) x out])
WOFF = {}
_c = 0
for _nm, _w in [("W1a", 128), ("W1b", 128), ("W1c", 128), ("W2", 128),
                ("W3", 128), ("W11a", 128), ("W11b", 128), ("W11c", 128),
                ("W12", 128), ("W13", 128), ("b13row", 128), ("Wdin", 512),
                ("Wdout0", 128), ("Wdout1", 128), ("Wdout2", 128), ("Wdout3", 128)]:
    WOFF[_nm] = _c
    _c += _w
WCOLS = _c

# bpack columns (f32): per-partition bias vectors
BOFF = {"b1": 0, "b2": 1, "b3s": 2, "b11": 3, "b12": 4, "bdout": 5,
        "bdin0": 6, "bdin1": 7, "bdin2": 8, "bdin3": 9}
NB = 10

# lnrep: replicated [128,128] f32 tiles: n1g n1b n2g n2b n3g n3b
LNOFF = {"n1g": 0, "n1b": 1, "n2g": 2, "n2b": 3, "n3g": 4, "n3b": 5}


def build_program(num_cores: int, nodes_total: int, trivial_ln3: bool,
                  trivial_ln12: bool, has_b13: bool = False):
    """Build the per-core Tile program. SPMD: all cores run the same code;
    per-core data differences come via in_maps."""
    nc = bacc.Bacc("TRN2", target_bir_lowering=False, debug=False,
                   num_devices=num_cores)
    NT = nodes_total
    NRANK = NT // 128          # table col-blocks

    d_hE32 = nc.dram_tensor("hE32", [128, E], F32, kind="ExternalInput").ap()
    d_hEbf = nc.dram_tensor("hEbf", [128, E], BF16, kind="ExternalInput").ap()
    d_hVfm32 = nc.dram_tensor("hVfm32", [128, PCN], F32, kind="ExternalInput").ap()
    d_hVfmbf = nc.dram_tensor("hVfmbf", [128, PCN], BF16, kind="ExternalInput").ap()
    d_table1 = nc.dram_tensor("table1", [128, NT], BF16, kind="ExternalInput").ap()
    d_idx = nc.dram_tensor("idx", [128, E // 16], I16, kind="ExternalInput").ap()
    d_wpack = nc.dram_tensor("wpack", [128, WCOLS], BF16, kind="ExternalInput").ap()
    d_bpack = nc.dram_tensor("bpack", [128, NB], F32, kind="ExternalInput").ap()
    d_lnrep = nc.dram_tensor("lnrep", [128, 6 * 128], F32, kind="ExternalInput").ap()

    import os
    DBG = bool(os.environ.get("MPNN_DEBUG"))
    if DBG:
        d_dbg_sg2 = nc.dram_tensor("dbg_sg2", [128, PCN], F32, kind="ExternalOutput").ap()
        d_dbg_r1 = nc.dram_tensor("dbg_r1", [128, PCN], F32, kind="ExternalOutput").ap()
        d_dbg_hv1 = nc.dram_tensor("dbg_hv1", [128, 4, 128], F32, kind="ExternalOutput").ap()
        d_dbg_r2 = nc.dram_tensor("dbg_r2", [128, PCN], F32, kind="ExternalOutput").ap()
        d_dbg_a2 = nc.dram_tensor("dbg_a2", [128, 768], F32, kind="ExternalOutput").ap()
        d_dbg_ps1 = nc.dram_tensor("dbg_ps1", [128, 768], F32, kind="ExternalOutput").ap()
        d_dbg_gnb = nc.dram_tensor("dbg_gnb", [128, 768], F32, kind="ExternalOutput").ap()
    d_hVout = nc.dram_tensor("hVout", [PCN, 128], F32, kind="ExternalOutput").ap()
    d_hEout = nc.dram_tensor("hEout", [E, 128], F32, kind="ExternalOutput").ap()

    multi = num_cores > 1
    if multi:
        d_ccin = nc.dram_tensor("cc_in", [PCN, 128], BF16).ap()
        d_ccout = nc.dram_tensor("cc_out", [num_cores * PCN, 128], BF16,
                                 addr_space="Shared").ap()

    with tile.TileContext(nc) as tc:
        with tc.tile_pool(name="const", bufs=1) as cp, \
             tc.tile_pool(name="work", bufs=2) as wp, \
             tc.tile_pool(name="small", bufs=2) as sp:

            # ---- gather-gating loads first ----
            idx = cp.tile([128, E // 16], I16)
            nc.sync.dma_start(out=idx[:], in_=d_idx[:])
            def gather_split(src_ap, tag, pool, nsp, dram=False):
                qt = E // nsp
                outs = []
                for q in range(nsp):
                    g = pool.tile([128, 1, qt], BF16, tag=f"{tag}{q % 2}")
                    kw = {} if dram else dict(
                        sbuf_tokens_per_rank=128, sbuf_free_dim_per_rank=256,
                        sbuf_free_dim_pad_per_rank=0, sbuf_byte_offset=0)
                    nc.gpsimd.dma_gather(
                        out_ap=g[:], in_ap=src_ap,
                        idxs_ap=idx[:, q * (qt // 16):(q + 1) * (qt // 16)],
                        num_idxs=qt, num_idxs_reg=qt, elem_size=128,
                        transpose=True, single_packet=False, **kw)
                    outs.append(g)
                return outs

            t1cm = tc.tile_pool(name="t1pool", bufs=1)
            t1pool = t1cm.__enter__()
            table1 = t1pool.tile([128, NT], BF16)
            nc.sync.dma_start(out=table1[:], in_=d_table1[:])
            EQ = E // 4
            hEbf_q = []
            for q in range(4):
                t = cp.tile([128, EQ], BF16, tag=f"hEbf{q}")
                nc.sync.dma_start(out=t[:], in_=d_hEbf[:, q * EQ:(q + 1) * EQ])
                hEbf_q.append(t)
            # ---- constants ----
            W = cp.tile([128, WCOLS], BF16)
            nc.sync.dma_start(out=W[:], in_=d_wpack[:])
            bia = cp.tile([128, NB], F32)
            nc.sync.dma_start(out=bia[:], in_=d_bpack[:])
            lnrep = cp.tile([128, 6 * 128], F32)
            nc.sync.dma_start(out=lnrep[:], in_=d_lnrep[:])
            hVfm32 = cp.tile([128, PCN], F32)
            nc.sync.dma_start(out=hVfm32[:], in_=d_hVfm32[:])
            hVfmbf = cp.tile([128, PCN], BF16)
            nc.sync.dma_start(out=hVfmbf[:], in_=d_hVfmbf[:])
            iden = cp.tile([128, 128], BF16)
            make_identity(nc, iden[:])
            iden32 = cp.tile([128, 128], F32)
            make_identity(nc, iden32[:])
            cen32 = cp.tile([128, 128], F32)
            nc.gpsimd.memset(cen32[:], -1.0 / 128.0)
            nc.gpsimd.affine_select(
                out=cen32[:], in_=cen32[:], compare_op=ALU.not_equal,
                fill=127.0 / 128.0, base=0, pattern=[[-1, 128]],
                channel_multiplier=1)
            ones_row = cp.tile([1, 512], BF16)
            nc.gpsimd.memset(ones_row[:], 1.0)
            eps_col = cp.tile([128, 1], F32)
            nc.gpsimd.memset(eps_col[:], EPS)

            def w(name):
                return W[:, WOFF[name]:WOFF[name] + (512 if name == "Wdin" else 128)]

            def bcol(name):
                return bia[:, BOFF[name]:BOFF[name] + 1]

            def rep(name):
                o = LNOFF[name] * 128
                return lnrep[:, o:o + 128]

            def bn_combine(pool, bn6, T, tagp):
                """bn_stats [128,T,6] -> (m, inv) [128,T] f32.
                var4 = (cve+cvo)/32 + (me-mo)^2 = 4*var;
                inv  = 1/sqrt(0.25*var4 + eps) via ACT scale."""
                d = pool.tile([128, T], F32, tag=f"{tagp}d")
                v = pool.tile([128, T], F32, tag=f"{tagp}v")
                inv = pool.tile([128, T], F32, tag=f"{tagp}i")
                nc.vector.tensor_tensor(out=d[:], in0=bn6[:, :, 1],
                                        in1=bn6[:, :, 4], op=ALU.subtract)
                nc.vector.tensor_tensor(out=d[:], in0=d[:], in1=d[:], op=ALU.mult)
                nc.vector.tensor_tensor(out=v[:], in0=bn6[:, :, 2],
                                        in1=bn6[:, :, 5], op=ALU.add)
                nc.vector.tensor_scalar(out=v[:], in0=v[:], scalar1=1.0 / 32.0,
                                        scalar2=None, op0=ALU.mult)
                nc.vector.tensor_tensor(out=v[:], in0=v[:], in1=d[:], op=ALU.add)
                nc.scalar.activation(v[:], v[:], AF.Sqrt, bias=eps_col[:, 0:1],
                                     scale=0.25)
                nc.vector.reciprocal(inv[:], v[:])
                return inv

            Sg2 = cp.tile([128, PCN], F32)   # sum_k gelu2 accumulator

            # ================= phase 1: node update =================
            with tc.tile_pool(name="p1gath", bufs=2) as gp1, \
                 tc.tile_pool(name="p1psA", bufs=2, space="PSUM") as psA, \
                 tc.tile_pool(name="p1psB", bufs=2, space="PSUM") as psB:

                NSP = 8
                gnb1 = gather_split(table1[:], "gnb1", gp1, NSP)

                for g in range(NCH // G1):
                    e0 = g * G1 * CH
                    n0 = e0 // Kn
                    nn = G1 * NPC
                    # bank-padded: each 384-wide chunk in its own 512-slot bank
                    ps1 = psA.tile([128, G1, 512], F32, tag="p1")
                    for c in range(G1):
                        eo = e0 + c * CH
                        qe, qeo = eo // (E // 4), eo % (E // 4)
                        nc.tensor.matmul(ps1[:, c, 0:CH], w("W1b"),
                                         hEbf_q[qe][:, qeo:qeo + CH],
                                         start=True, stop=False)
                    for c in range(G1):
                        eo = e0 + c * CH
                        q, qo = eo // QT, eo % QT
                        nc.tensor.matmul(ps1[:, c, 0:CH], w("W1c"),
                                         gnb1[q][:, 0, qo:qo + CH],
                                         start=False, stop=False)
                    for c in range(G1):
                        nc.tensor.matmul(
                            ps1[:, c, 0:CH].rearrange("p (n k) -> p n k", k=Kn),
                            w("W1a"),
                            hVfmbf[:, n0 + c * NPC:n0 + (c + 1) * NPC]
                            .to_broadcast([128, NPC, Kn]),
                            start=False, stop=True)
                    if DBG and g == 0:
                        dbg_ps1 = wp.tile([128, 768], F32, tag="dbgps1")
                        nc.vector.tensor_copy(
                            dbg_ps1[:].rearrange("p (c e) -> p c e", c=G1),
                            ps1[:, :, 0:CH])
                        nc.sync.dma_start(out=d_dbg_ps1[:], in_=dbg_ps1[:])
                        dbg_gnb = wp.tile([128, 768], F32, tag="dbggnb")
                        nc.vector.tensor_copy(dbg_gnb[:], gnb1[0][:, 0, 0:768])
                        nc.sync.dma_start(out=d_dbg_gnb[:], in_=dbg_gnb[:])
                    a1 = wp.tile([128, G1 * CH], BF16, tag="a1")
                    nc.scalar.activation(
                        a1[:].rearrange("p (c e) -> p c e", c=G1),
                        ps1[:, :, 0:CH], AF.Gelu, bias=bcol("b1"))
                    ps2 = psB.tile([128, G1, 512], F32, tag="p2")
                    for c in range(G1):
                        nc.tensor.matmul(ps2[:, c, 0:CH], w("W2"),
                                         a1[:, c * CH:(c + 1) * CH],
                                         start=True, stop=True)
                    a2 = wp.tile([128, G1 * CH], BF16, tag="a2")
                    nc.scalar.activation(
                        a2[:].rearrange("p (c e) -> p c e", c=G1),
                        ps2[:, :, 0:CH], AF.Gelu, bias=bcol("b2"))
                    nc.vector.tensor_reduce(
                        Sg2[:, n0:n0 + nn],
                        a2[:].rearrange("p (n k) -> p n k", k=Kn),
                        axis=AX.X, op=ALU.add)
                    if DBG and g == 0:
                        dbg_a2f = wp.tile([128, 768], F32, tag="dbga2")
                        nc.vector.tensor_copy(dbg_a2f[:], a2[:])
                        nc.sync.dma_start(out=d_dbg_a2[:], in_=dbg_a2f[:])

            t1cm.__exit__(None, None, None)

            # ---- node path: dh, LN1, FFN, LN2 ----
            def ln_rowmajor_nodes(src_fm, gname, bname, out_rm_sb, pspool):
                ps_rm = pspool.tile([128, 4, 128], F32, tag="nrm")
                for j in range(4):
                    nc.tensor.matmul(ps_rm[:, j, :],
                                     src_fm[:, j * 128:(j + 1) * 128],
                                     cen32[:], start=True, stop=True)
                bn6 = sp.tile([128, 4, 8], F32, tag="nbn6")
                for j in range(4):
                    nc.vector.bn_stats(bn6[:, j, 0:6], ps_rm[:, j, :])
                inv = bn_combine(sp, bn6, 4, "n")
                for j in range(4):
                    nc.vector.tensor_scalar(
                        out=out_rm_sb[:, j, :], in0=ps_rm[:, j, :],
                        scalar1=inv[:, j:j + 1], scalar2=None, op0=ALU.mult)
                if not trivial_ln12:
                    nc.vector.tensor_tensor(
                        out=out_rm_sb[:], in0=out_rm_sb[:],
                        in1=rep(gname)[:].to_broadcast([128, 128, 4]).rearrange(
                            "p h j -> p j h"),
                        op=ALU.mult)
                    nc.vector.tensor_tensor(
                        out=out_rm_sb[:], in0=out_rm_sb[:],
                        in1=rep(bname)[:].to_broadcast([128, 128, 4]).rearrange(
                            "p h j -> p j h"),
                        op=ALU.add)

            with tc.tile_pool(name="npsA", bufs=1, space="PSUM") as npsA, \
                 tc.tile_pool(name="npsB", bufs=2, space="PSUM") as npsB:
                Sg2bf = cp.tile([128, PCN], BF16)
                nc.vector.tensor_copy(Sg2bf[:], Sg2[:])
                ps_dh = npsA.tile([128, PCN], F32, tag="pdh")
                nc.tensor.matmul(ps_dh[:], w("W3"), Sg2bf[:], start=True, stop=True)
                r1 = cp.tile([128, PCN], F32)
                nc.vector.tensor_scalar(out=r1[:], in0=ps_dh[:],
                                        scalar1=1.0 / SCALE, scalar2=bcol("b3s"),
                                        op0=ALU.mult, op1=ALU.add)
                nc.vector.tensor_tensor(out=r1[:], in0=r1[:], in1=hVfm32[:],
                                        op=ALU.add)

                hV1_rm = cp.tile([128, 4, 128], F32)
                ln_rowmajor_nodes(r1, "n1g", "n1b", hV1_rm, npsA)
                if DBG:
                    nc.sync.dma_start(out=d_dbg_sg2[:], in_=Sg2[:])
                    nc.sync.dma_start(out=d_dbg_r1[:], in_=r1[:])
                    nc.sync.dma_start(out=d_dbg_hv1[:], in_=hV1_rm[:])

                ps_fm = npsA.tile([128, PCN], F32, tag="nfm")
                for j in range(4):
                    nc.tensor.transpose(ps_fm[:, j * 128:(j + 1) * 128],
                                        hV1_rm[:, j, :], iden32[:])
                hV1fm32 = cp.tile([128, PCN], F32)
                nc.vector.tensor_copy(hV1fm32[:], ps_fm[:])
                hV1fmbf = cp.tile([128, PCN], BF16)
                nc.vector.tensor_copy(hV1fmbf[:], ps_fm[:])

                af = []
                for j in range(4):
                    psf = npsB.tile([128, PCN], F32, tag="ffn1")
                    nc.tensor.matmul(psf[:], w("Wdin")[:, j * 128:(j + 1) * 128],
                                     hV1fmbf[:], start=True, stop=True)
                    a = wp.tile([128, PCN], BF16, tag=f"af{j}")
                    nc.scalar.activation(a[:], psf[:], AF.Gelu,
                                         bias=bcol(f"bdin{j}"))
                    af.append(a)
                ps_o = npsA.tile([128, PCN], F32, tag="ffn2")
                for j in range(4):
                    nc.tensor.matmul(ps_o[:], w(f"Wdout{j}"), af[j][:],
                                     start=(j == 0), stop=(j == 3))
                r2 = cp.tile([128, PCN], F32)
                nc.vector.tensor_scalar_add(r2[:], ps_o[:], bcol("bdout"))
                nc.vector.tensor_tensor(out=r2[:], in0=r2[:], in1=hV1fm32[:],
                                        op=ALU.add)

                if DBG:
                    nc.sync.dma_start(out=d_dbg_r2[:], in_=r2[:])
                hV2_rm = cp.tile([128, 4, 128], F32)
                ln_rowmajor_nodes(r2, "n2g", "n2b", hV2_rm, npsA)

                nc.sync.dma_start(
                    out=d_hVout[:].rearrange("(t p) h -> p t h", p=128),
                    in_=hV2_rm[:])

                ps_fm2 = npsA.tile([128, PCN], F32, tag="nfm")
                for j in range(4):
                    nc.tensor.transpose(ps_fm2[:, j * 128:(j + 1) * 128],
                                        hV2_rm[:, j, :], iden32[:])
                hV2fmbf = cp.tile([128, PCN], BF16)
                nc.vector.tensor_copy(hV2fmbf[:], ps_fm2[:])

                tslice = cp.tile([128, 4, 128], BF16)
                nc.vector.tensor_copy(tslice[:], hV2_rm[:])

            # ---- phase-2 gather table via AllGather (row-major DRAM) ----
            if multi:
                nc.sync.dma_start(
                    out=d_ccin[:].rearrange("(t p) h -> p t h", p=128),
                    in_=tslice[:])
                nc.gpsimd.collective_compute(
                    "AllGather", ALU.bypass,
                    ins=[d_ccin[:]], outs=[d_ccout[:]],
                    replica_groups=[list(range(num_cores))])
                d_table2 = d_ccout
            else:
                d_table2 = nc.dram_tensor("table2d", [PCN, 128], BF16).ap()
                nc.sync.dma_start(
                    out=d_table2[:].rearrange("(t p) h -> p t h", p=128),
                    in_=tslice[:])

            # ================= phase 2: edge update =================
            with tc.tile_pool(name="p2gath", bufs=2) as gp2, \
                 tc.tile_pool(name="p2work", bufs=2) as wp2, \
                 tc.tile_pool(name="psq1", bufs=2, space="PSUM") as psq1, \
                 tc.tile_pool(name="psq2", bufs=2, space="PSUM") as psq2, \
                 tc.tile_pool(name="psq3", bufs=2, space="PSUM") as psq3, \
                 tc.tile_pool(name="psq4", bufs=2, space="PSUM") as psq4:

                gnb2 = gather_split(d_table2[:], "gnb2", gp2, NSP, dram=True)

                GB = 8                      # chunks per tail batch
                NT3 = 3 * GB                # rm tiles per batch
                for gb in range(NCH // GB):
                    hE32g = wp2.tile([128, GB * CH], F32, tag="he32")
                    e00 = gb * GB * CH
                    nc.sync.dma_start(out=hE32g[:],
                                      in_=d_hE32[:, e00:e00 + GB * CH])
                    rm_sb = wp2.tile([128, NT3, 128], F32, tag="qrmsb")
                    for ci in range(GB):
                        c = gb * GB + ci
                        e0 = c * CH
                        eo = ci * CH
                        n0 = e0 // Kn
                        q, qo = e0 // QT, e0 % QT
                        qe, qeo = e0 // (E // 4), e0 % (E // 4)
                        ps1 = psq1.tile([128, CH], F32, tag="q1")
                        nc.tensor.matmul(ps1[:], w("W11b"),
                                         hEbf_q[qe][:, qeo:qeo + CH],
                                         start=True, stop=False)
                        nc.tensor.matmul(ps1[:], w("W11c"),
                                         gnb2[q][:, 0, qo:qo + CH],
                                         start=False, stop=False)
                        nc.tensor.matmul(
                            ps1[:].rearrange("p (n k) -> p n k", k=Kn), w("W11a"),
                            hV2fmbf[:, n0:n0 + NPC].to_broadcast([128, NPC, Kn]),
                            start=False, stop=True)
                        a1 = wp.tile([128, CH], BF16, tag="qa1")
                        nc.scalar.activation(a1[:], ps1[:], AF.Gelu,
                                             bias=bcol("b11"))
                        ps2 = psq2.tile([128, CH], F32, tag="q2")
                        nc.tensor.matmul(ps2[:], w("W12"), a1[:],
                                         start=True, stop=True)
                        a2 = wp.tile([128, CH], BF16, tag="qa2")
                        nc.scalar.activation(a2[:], ps2[:], AF.Gelu,
                                             bias=bcol("b12"))
                        ps3 = psq3.tile([128, CH], F32, tag="q3")
                        nc.tensor.matmul(ps3[:], w("W13"), a2[:], start=True,
                                         stop=not has_b13)
                        if has_b13:
                            nc.tensor.matmul(
                                ps3[:],
                                W[0:1, WOFF["b13row"]:WOFF["b13row"] + 128],
                                ones_row[0:1, 0:CH], start=False, stop=True)

                        r = wp.tile([128, CH], F32, tag="qr")
                        nc.vector.tensor_tensor(out=r[:], in0=ps3[:],
                                                in1=hE32g[:, eo:eo + CH],
                                                op=ALU.add)
                        ps_rm = psq4.tile([128, 3, 128], F32, tag="qrm")
                        for t in range(3):
                            nc.tensor.matmul(ps_rm[:, t, :],
                                             r[:, t * 128:(t + 1) * 128],
                                             cen32[:], start=True, stop=True)
                        # copy rm to SBUF so psum frees and sqrt can batch
                        nc.scalar.copy(rm_sb[:, 3 * ci:3 * ci + 3, :], ps_rm[:])
                    bn6 = sp.tile([128, NT3, 8], F32, tag="qbn6")
                    for t in range(NT3):
                        nc.vector.bn_stats(bn6[:, t, 0:6], rm_sb[:, t, :])
                    inv = bn_combine(sp, bn6, NT3, "q")
                    out_rm = rm_sb
                    nc.vector.tensor_tensor(
                        out=out_rm[:], in0=rm_sb[:],
                        in1=inv[:].to_broadcast([128, NT3, 128]), op=ALU.mult)
                    if not trivial_ln3:
                        nc.vector.tensor_tensor(
                            out=out_rm[:], in0=out_rm[:],
                            in1=rep("n3g")[:].to_broadcast([128, 128, NT3])
                            .rearrange("p h t -> p t h"),
                            op=ALU.mult)
                        nc.vector.tensor_tensor(
                            out=out_rm[:], in0=out_rm[:],
                            in1=rep("n3b")[:].to_broadcast([128, 128, NT3])
                            .rearrange("p h t -> p t h"),
                            op=ALU.add)
                    nc.sync.dma_start(
                        out=d_hEout[e00:e00 + GB * CH, :]
                        .rearrange("(t p) h -> p t h", p=128),
                        in_=out_rm[:])

    nc.compile()
    return nc


_PROG_CACHE = {}


def _get_prog(num_cores, nodes_total, trivial_ln3, trivial_ln12, has_b13):
    key = (num_cores, nodes_total, trivial_ln3, trivial_ln12, has_b13)
    if key not in _PROG_CACHE:
        _PROG_CACHE[key] = build_program(num_cores, nodes_total, trivial_ln3,
                                         trivial_ln12, has_b13)
    return _PROG_CACHE[key]


def make_in_maps(inputs, num_cores=CORES):
    """Host-side sharding/marshaling: layout transforms and casts only."""
    nodes = np.asarray(inputs["h_V"]).reshape(-1, H).shape[0]
    hV = np.asarray(inputs["h_V"], np.float32).reshape(nodes, H)
    hE = np.asarray(inputs["h_E"], np.float32).reshape(nodes, Kn, H)
    Eidx = np.asarray(inputs["E_idx"])
    bsz = Eidx.shape[0]
    npb = Eidx.shape[1]
    gidx = (Eidx.reshape(bsz, npb, Kn)
            + (np.arange(bsz) * npb)[:, None, None]).reshape(nodes, Kn)
    gidx = gidx.astype(np.int16)

    assert np.all(np.asarray(inputs["mask_V"]) == 1.0), "kernel assumes mask_V==1"
    assert np.all(np.asarray(inputs["mask_attend"]) == 1.0), \
        "kernel assumes mask_attend==1"

    W1 = np.asarray(inputs["W1"], np.float32)
    W11 = np.asarray(inputs["W11"], np.float32)
    wcols = np.zeros((128, WCOLS), np.float32)
    wcols[:, WOFF["W1a"]:WOFF["W1a"] + 128] = W1[0:128]
    wcols[:, WOFF["W1b"]:WOFF["W1b"] + 128] = W1[128:256]
    wcols[:, WOFF["W1c"]:WOFF["W1c"] + 128] = W1[256:384]
    wcols[:, WOFF["W2"]:WOFF["W2"] + 128] = np.asarray(inputs["W2"], np.float32)
    wcols[:, WOFF["W3"]:WOFF["W3"] + 128] = np.asarray(inputs["W3"], np.float32)
    wcols[:, WOFF["W11a"]:WOFF["W11a"] + 128] = W11[0:128]
    wcols[:, WOFF["W11b"]:WOFF["W11b"] + 128] = W11[128:256]
    wcols[:, WOFF["W11c"]:WOFF["W11c"] + 128] = W11[256:384]
    wcols[:, WOFF["W12"]:WOFF["W12"] + 128] = np.asarray(inputs["W12"], np.float32)
    wcols[:, WOFF["W13"]:WOFF["W13"] + 128] = np.asarray(inputs["W13"], np.float32)
    wcols[:, WOFF["Wdin"]:WOFF["Wdin"] + 512] = np.asarray(inputs["Wd_in"],
                                                           np.float32)
    wcols[0, WOFF["b13row"]:WOFF["b13row"] + 128] = np.asarray(inputs["b13"],
                                                               np.float32)
    Wd_out = np.asarray(inputs["Wd_out"], np.float32)
    for j in range(4):
        wcols[:, WOFF[f"Wdout{j}"]:WOFF[f"Wdout{j}"] + 128] = \
            Wd_out[j * 128:(j + 1) * 128]
    wpack = wcols.astype(bf16)

    bp = np.zeros((128, NB), np.float32)
    bp[:, BOFF["b1"]] = np.asarray(inputs["b1"], np.float32)
    bp[:, BOFF["b2"]] = np.asarray(inputs["b2"], np.float32)
    bp[:, BOFF["b3s"]] = np.asarray(inputs["b3"], np.float32) * Kn / SCALE
    bp[:, BOFF["b11"]] = np.asarray(inputs["b11"], np.float32)
    bp[:, BOFF["b12"]] = np.asarray(inputs["b12"], np.float32)
    bp[:, BOFF["bdout"]] = np.asarray(inputs["bd_out"], np.float32)
    bd_in = np.asarray(inputs["bd_in"], np.float32)
    for j in range(4):
        bp[:, BOFF[f"bdin{j}"]] = bd_in[j * 128:(j + 1) * 128]

    lnrep = np.zeros((128, 6 * 128), np.float32)
    for nm in ["n1g", "n1b", "n2g", "n2b", "n3g", "n3b"]:
        o = LNOFF[nm] * 128
        lnrep[:, o:o + 128] = np.asarray(inputs[nm], np.float32)[None, :]

    # phase-1 gather table: token t -> partition t%128, col block t//128
    table1 = hV.astype(bf16).reshape(nodes // 128, 128, H) \
        .transpose(1, 0, 2).reshape(128, nodes)

    in_maps = []
    pcn = nodes // num_cores
    for c in range(num_cores):
        n0 = c * pcn
        sl = slice(n0, n0 + pcn)
        hE_fm = np.ascontiguousarray(hE[sl].reshape(pcn * Kn, H).T)
        hv_slice = hV[sl].T
        idx_flat = gidx[sl].reshape(-1)
        idx_w = idx_flat.reshape(-1, 16).T
        idx_rep = np.tile(idx_w, (8, 1))
        in_maps.append({
            "hE32": hE_fm,
            "hEbf": hE_fm.astype(bf16),
            "hVfm32": np.ascontiguousarray(hv_slice, np.float32),
            "hVfmbf": np.ascontiguousarray(hv_slice).astype(bf16),
            "table1": np.ascontiguousarray(table1),
            "idx": np.ascontiguousarray(idx_rep),
            "wpack": wpack,
            "bpack": bp,
            "lnrep": lnrep,
        })
    return in_maps


def _trivial_flags(inputs):
    t3 = (np.all(np.asarray(inputs["n3g"]) == 1.0)
          and np.all(np.asarray(inputs["n3b"]) == 0.0))
    t12 = (np.all(np.asarray(inputs["n1g"]) == 1.0)
           and np.all(np.asarray(inputs["n1b"]) == 0.0)
           and np.all(np.asarray(inputs["n2g"]) == 1.0)
           and np.all(np.asarray(inputs["n2b"]) == 0.0))
    return t3, t12


def _run(inputs, trace=False):
    t3, t12 = _trivial_flags(inputs)
    has_b13 = bool(np.any(np.asarray(inputs["b13"]) != 0.0))
    nc = _get_prog(CORES, NODES, t3, t12, has_b13)
    in_maps = make_in_maps(inputs, CORES)
    res = run_bass_kernel_spmd(nc, in_maps, list(range(CORES)), trace=trace)
    hV_out = np.concatenate([res.results[c]["hVout"] for c in range(CORES)], 0)
    hE_out = np.concatenate([res.results[c]["hEout"] for c in range(CORES)], 0)
    out = (hV_out.reshape(B, N, H), hE_out.reshape(B, N, Kn, H))
    return out, res


def kernel(**inputs):
    return _run(inputs, trace=False)[0]
